# revision 3
# baseline (speedup 1.0000x reference)
"""DiskKinematics histogram-binning kernel for 8x TRN2 NeuronCores.

Strategy (data-parallel over particles, 2M particles/core):
  - Host converts inputs to bf16 and packs planar [T, 128, 6G] per core
    (planes x, y, vx, vy, vz, m; the unused position z is dropped), so
    DMA traffic is 24MB/core and DVE ops run in 2x/4x half-precision
    modes.
  - Radial bin index idx = floor(5r) is radix-decomposed as idx = 3c + f
    (c in 0..16 coarse, f in 0..2 fine). Bin keys are built with the
    bf16 magic-number trick (x + 191.5 rounds to 192 + floor(x)).
  - Weighted moment features f1..f6 are computed with wide bf16
    tensor_tensor ops into planes adjacent to m, giving a 7-plane
    [m f1..f6] stationary view.
  - Scatter: per sub-batch, one wide is_equal builds fine one-hots
    (3/particle) and coarse one-hots (17/particle); one wide multiply
    expands E = oh_f (x) wf (21/particle). Per 128-particle group a
    single PE matmul accumulates psum[17, 21] += oh_c.T @ E.
  - Host sums per-core psums, remaps (c, f) -> 3c+f, drops virtual bin
    50, rescales moments (the kernel accumulates vr/5, vr^2/25 etc.),
    and computes means/sigmas in f64.
"""

import numpy as np
import ml_dtypes

import concourse.bass as bass
import concourse.bacc as bacc
import concourse.mybir as mybir
from concourse.tile import TileContext
from concourse.bass_utils import run_bass_kernel_spmd

P = 128
N_CORES = 8
A = 3              # fine radix
CB = 17            # coarse bins (A*CB = 51 virtual bins; idx 50 dropped)
F = 7              # m + 6 weighted moments
G = 625            # particles per partition per tile
NB = 5             # sub-batches per tile (for one-hot/E tiles)
B = G // NB
NPLANES = 12       # x y vx vy vz m f1..f6
KEY0 = 192.0       # bf16 magic base: [128,256) has spacing 1.0

f32 = mybir.dt.float32
bf16 = mybir.dt.bfloat16

_CACHE = {}


def _build(n_tiles: int, reps: int = 1):
    nc = bacc.Bacc(None, target_bir_lowering=False, debug=False)
    data = nc.dram_tensor("data", [n_tiles, P, 6 * G], bf16, kind="ExternalInput")
    iotas = nc.dram_tensor("iotas", [P, (A + CB) * B], bf16, kind="ExternalInput")
    hist = nc.dram_tensor("hist", [CB, A * F], f32, kind="ExternalOutput")

    AO = mybir.AluOpType
    AF = mybir.ActivationFunctionType

    n_mm = reps * n_tiles * G

    with TileContext(nc) as tc:
        with (
            tc.tile_pool(name="io", bufs=2) as iop,
            tc.tile_pool(name="fp", bufs=2) as fpp,
            tc.tile_pool(name="ar", bufs=2) as arp,
            tc.tile_pool(name="oh", bufs=2) as ohp,
            tc.tile_pool(name="cst", bufs=1) as cst,
            tc.tile_pool(name="ps", bufs=1, space="PSUM") as psp,
        ):
            iota_t = cst.tile([P, (A + CB) * B], bf16)
            nc.sync.dma_start(out=iota_t[:], in_=iotas[:])
            iota_f = iota_t[:, 0 : A * B].rearrange("p (a g) -> p a g", a=A)
            iota_c = iota_t[:, A * B :].rearrange("p (c g) -> p c g", c=CB)

            ps = psp.tile([CB, A * F], f32)
            gi = 0

            for rep in range(reps):
                for t in range(n_tiles):
                    C = iop.tile([P, NPLANES * G], bf16, tag="C")
                    nc.sync.dma_start(out=C[:, 0 : 6 * G], in_=data[t])

                    def pl(i):
                        return C[:, i * G : (i + 1) * G]

                    x, y, vx, vy, vz, m = (pl(i) for i in range(6))

                    xx = fpp.tile([P, G], f32, tag="xx")
                    yy = fpp.tile([P, G], f32, tag="yy")
                    rsq = fpp.tile([P, G], f32, tag="rsq")
                    rcp = fpp.tile([P, G], f32, tag="rcp")
                    t5b = arp.tile([P, G], bf16, tag="t5b")
                    i5 = arp.tile([P, G], bf16, tag="i5")
                    vk = arp.tile([P, G], bf16, tag="vk")
                    ck = arp.tile([P, G], bf16, tag="ck")
                    t1 = arp.tile([P, G], bf16, tag="t1")
                    fk = arp.tile([P, G], bf16, tag="fk")
                    aa = arp.tile([P, G], bf16, tag="aa")
                    bb = arp.tile([P, G], bf16, tag="bb")
                    cc = arp.tile([P, G], bf16, tag="cc")
                    dd = arp.tile([P, G], bf16, tag="dd")
                    s = arp.tile([P, G], bf16, tag="s")
                    tt = arp.tile([P, G], bf16, tag="tt")
                    mi = arp.tile([P, G], bf16, tag="mi")
                    si = arp.tile([P, G], bf16, tag="si")
                    ti = arp.tile([P, G], bf16, tag="ti")

                    # --- binning keys -------------------------------------
                    nc.scalar.activation(xx[:], x, AF.Square)
                    nc.scalar.activation(yy[:], y, AF.Square)
                    nc.vector.tensor_tensor(out=rsq[:], in0=xx[:], in1=yy[:], op=AO.add)
                    nc.scalar.activation(t5b[:], rsq[:], AF.Sqrt, scale=25.0)
                    nc.vector.reciprocal_approx_fast(out=rcp[:], in_=rsq[:])
                    nc.scalar.activation(i5[:], rcp[:], AF.Sqrt, scale=0.04)
                    # vk = 192 + idx, ck = 192 + c, fk = 192 + idx - 3c
                    nc.vector.tensor_scalar(vk[:], t5b[:], KEY0 - 0.5, None, AO.add)
                    nc.vector.tensor_scalar(
                        ck[:], t5b[:], 1.0 / 3.0, KEY0 - 0.5, AO.mult, AO.add
                    )
                    nc.vector.tensor_scalar(
                        t1[:], ck[:], -3.0, 3.0 * KEY0, AO.mult, AO.add
                    )
                    nc.vector.tensor_tensor(out=fk[:], in0=vk[:], in1=t1[:], op=AO.add)

                    # --- velocity moments (bf16, scales folded on host) ---
                    nc.vector.tensor_tensor(out=aa[:], in0=x, in1=vx, op=AO.mult)
                    nc.vector.tensor_tensor(out=bb[:], in0=y, in1=vy, op=AO.mult)
                    nc.vector.tensor_tensor(out=cc[:], in0=y, in1=vx, op=AO.mult)
                    nc.vector.tensor_tensor(out=dd[:], in0=x, in1=vy, op=AO.mult)
                    nc.vector.tensor_tensor(out=s[:], in0=aa[:], in1=bb[:], op=AO.add)
                    nc.vector.tensor_tensor(out=tt[:], in0=cc[:], in1=dd[:], op=AO.subtract)
                    nc.vector.tensor_tensor(out=mi[:], in0=m, in1=i5[:], op=AO.mult)
                    nc.vector.tensor_tensor(out=si[:], in0=s[:], in1=i5[:], op=AO.mult)
                    nc.vector.tensor_tensor(out=ti[:], in0=tt[:], in1=i5[:], op=AO.mult)
                    nc.vector.tensor_tensor(out=pl(6), in0=s[:], in1=mi[:], op=AO.mult)
                    nc.vector.tensor_tensor(out=pl(7), in0=pl(6), in1=si[:], op=AO.mult)
                    nc.vector.tensor_tensor(out=pl(8), in0=tt[:], in1=mi[:], op=AO.mult)
                    nc.vector.tensor_tensor(out=pl(9), in0=pl(8), in1=ti[:], op=AO.mult)
                    nc.vector.tensor_tensor(out=pl(10), in0=m, in1=vz, op=AO.mult)
                    nc.vector.tensor_tensor(out=pl(11), in0=pl(10), in1=vz, op=AO.mult)

                    wf = C[:, 5 * G : 12 * G].rearrange("p (f g) -> p f g", f=F)

                    # --- scatter ------------------------------------------
                    for j in range(NB):
                        gs = slice(j * B, (j + 1) * B)
                        ohf = ohp.tile([P, A * B], bf16, tag="ohf")
                        ohc = ohp.tile([P, CB * B], bf16, tag="ohc")
                        E = ohp.tile([P, A * F * B], bf16, tag="E")

                        fkb = fk[:, gs].unsqueeze(1).broadcast_to([P, A, B])
                        nc.vector.tensor_tensor(
                            out=ohf[:].rearrange("p (a g) -> p a g", a=A),
                            in0=fkb, in1=iota_f, op=AO.is_equal,
                        )
                        ckb = ck[:, gs].unsqueeze(1).broadcast_to([P, CB, B])
                        nc.vector.tensor_tensor(
                            out=ohc[:].rearrange("p (c g) -> p c g", c=CB),
                            in0=ckb, in1=iota_c, op=AO.is_equal,
                        )
                        wfb = wf[:, :, gs].unsqueeze(1).broadcast_to([P, A, F, B])
                        ohfb = (
                            ohf[:].rearrange("p (a g) -> p a g", a=A)
                            .unsqueeze(2).broadcast_to([P, A, F, B])
                        )
                        nc.vector.tensor_tensor(
                            out=E[:].rearrange("p (a f g) -> p a f g", a=A, f=F),
                            in0=wfb, in1=ohfb, op=AO.mult,
                        )

                        ohc_v = ohc[:].rearrange("p (c g) -> p c g", c=CB)
                        E_v = E[:].rearrange("p (x g) -> p x g", x=A * F)
                        for g in range(B):
                            nc.tensor.matmul(
                                out=ps[:],
                                lhsT=ohc_v[:, :, g],
                                rhs=E_v[:, :, g],
                                start=(gi == 0),
                                stop=(gi == n_mm - 1),
                            )
                            gi += 1

            out_sb = cst.tile([CB, A * F], f32)
            nc.vector.tensor_copy(out=out_sb[:], in_=ps[:])
            nc.sync.dma_start(out=hist[:], in_=out_sb[:])

    nc.compile()
    return nc


LAST_RESULTS = None


def _pack_core(positions, velocities, masses, n_tiles):
    """f32 [npc,3]x2 + [npc] -> bf16 [T, 128, 6G] planar (x y vx vy vz m)."""
    out = np.empty((n_tiles, P, 6 * G), dtype=ml_dtypes.bfloat16)
    pr = positions.reshape(n_tiles, P, G, 3)
    vr = velocities.reshape(n_tiles, P, G, 3)
    out[:, :, 0 * G : 1 * G] = pr[:, :, :, 0]
    out[:, :, 1 * G : 2 * G] = pr[:, :, :, 1]
    out[:, :, 2 * G : 3 * G] = vr[:, :, :, 0]
    out[:, :, 3 * G : 4 * G] = vr[:, :, :, 1]
    out[:, :, 4 * G : 5 * G] = vr[:, :, :, 2]
    out[:, :, 5 * G : 6 * G] = masses.reshape(n_tiles, P, G)
    return out


def _iotas():
    it = np.empty(((A + CB) * B,), dtype=np.float32)
    for a in range(A):
        it[a * B : (a + 1) * B] = KEY0 + a
    for c in range(CB):
        it[(A + c) * B : (A + c + 1) * B] = KEY0 + c
    return np.tile(it[None, :], (P, 1)).astype(ml_dtypes.bfloat16)


def _postprocess(hsum):
    """hsum [CB, A*F] f64 -> kin [6, 50] f32."""
    h = hsum.reshape(CB, A, F).reshape(CB * A, F)[: A * CB - 1]  # drop idx 50
    mass = h[:, 0]
    with np.errstate(divide="ignore", invalid="ignore"):
        mm = h[:, 1:] / mass[:, None]
        vr_m = 5.0 * mm[:, 0]
        vr2 = 25.0 * mm[:, 1]
        vph_m = 5.0 * mm[:, 2]
        vph2 = 25.0 * mm[:, 3]
        vz_m = mm[:, 4]
        vz2 = mm[:, 5]
        vr_sig = np.sqrt(np.maximum(vr2 - vr_m**2, 0.0))
        vph_sig = np.sqrt(np.maximum(vph2 - vph_m**2, 0.0))
        vz_sig = np.sqrt(np.maximum(vz2 - vz_m**2, 0.0))
    return np.stack((vph_m, vph_sig, vr_m, vr_sig, vz_m, vz_sig)).astype(np.float32)


def kernel(positions, velocities, masses, trace=False):
    global LAST_RESULTS
    positions = np.ascontiguousarray(np.asarray(positions, dtype=np.float32))
    velocities = np.ascontiguousarray(np.asarray(velocities, dtype=np.float32))
    masses = np.ascontiguousarray(np.asarray(masses, dtype=np.float32))
    n = positions.shape[0]
    assert n % (N_CORES * P * G) == 0, n
    npc = n // N_CORES
    n_tiles = npc // (P * G)

    key = n_tiles
    if key not in _CACHE:
        _CACHE[key] = _build(n_tiles)
    nc = _CACHE[key]

    iota = _iotas()
    in_maps = []
    for k in range(N_CORES):
        sl = slice(k * npc, (k + 1) * npc)
        in_maps.append(
            {
                "data": _pack_core(
                    positions[sl], velocities[sl], masses[sl], n_tiles
                ),
                "iotas": iota,
            }
        )

    res = run_bass_kernel_spmd(
        nc, in_maps, core_ids=list(range(N_CORES)), trace=trace
    )
    LAST_RESULTS = res

    hsum = np.zeros((CB, A * F), dtype=np.float64)
    for r in res.results:
        hsum += r["hist"].astype(np.float64)
    return _postprocess(hsum)


# revision 7
# speedup vs baseline: 1.2442x; 1.2442x over previous
"""DiskKinematics histogram-binning kernel for 8x TRN2 NeuronCores.

Strategy (data-parallel over particles, 2M particles/core):
  - Host converts inputs to bf16 and packs planar [T, 128, 6G] per core
    (planes x, y, vx, vy, vz, m; the unused position z is dropped), so
    DMA traffic is 24MB/core and DVE ops run in 2x/4x half-precision
    modes.
  - Radial bin index idx = floor(5r) is radix-decomposed as idx = 3c + f
    (c in 0..16 coarse, f in 0..2 fine). Bin keys are built with the
    bf16 magic-number trick (x + 191.5 rounds to 192 + floor(x)).
  - Weighted moment features f1..f6 are computed with wide bf16
    tensor_tensor ops into planes adjacent to m, giving a 7-plane
    [m f1..f6] stationary view.
  - Scatter: per sub-batch, one wide is_equal builds fine one-hots
    (3/particle) and coarse one-hots (17/particle); one wide multiply
    expands E = oh_f (x) wf (21/particle). Per 128-particle group a
    single PE matmul accumulates psum[17, 21] += oh_c.T @ E.
  - Host sums per-core psums, remaps (c, f) -> 3c+f, drops virtual bin
    50, rescales moments (the kernel accumulates vr/5, vr^2/25 etc.),
    and computes means/sigmas in f64.
"""

import numpy as np
import ml_dtypes

import concourse.bass as bass
import concourse.bacc as bacc
import concourse.mybir as mybir
from concourse.tile import TileContext
from concourse.bass_utils import run_bass_kernel_spmd

P = 128
N_CORES = 8
A = 3              # fine radix
CB = 17            # coarse bins (A*CB = 51 virtual bins; idx 50 dropped)
F = 7              # m + 6 weighted moments
G = 625            # particles per partition per tile
NB = 5             # sub-batches per tile (for one-hot/E tiles)
B = G // NB
NPLANES = 12       # x y vx vy vz m f1..f6
KEY0 = 192.0       # bf16 magic base: [128,256) has spacing 1.0

f32 = mybir.dt.float32
bf16 = mybir.dt.bfloat16

_CACHE = {}


def _build(n_tiles: int, reps: int = 1):
    nc = bacc.Bacc(None, target_bir_lowering=False, debug=False)
    data = nc.dram_tensor("data", [n_tiles, P, 6 * G], bf16, kind="ExternalInput")
    iotas = nc.dram_tensor("iotas", [P, (A + CB) * B], bf16, kind="ExternalInput")
    hist = nc.dram_tensor("hist", [CB, A * F], f32, kind="ExternalOutput")

    AO = mybir.AluOpType
    AF = mybir.ActivationFunctionType

    n_mm = n_tiles * G

    with TileContext(nc) as tc:
        with (
            tc.tile_pool(name="io", bufs=2) as iop,
            tc.tile_pool(name="fp", bufs=2) as fpp,
            tc.tile_pool(name="ar", bufs=2) as arp,
            tc.tile_pool(name="oh", bufs=2) as ohp,
            tc.tile_pool(name="cst", bufs=1) as cst,
            tc.tile_pool(name="ps", bufs=1, space="PSUM") as psp,
        ):
            iota_t = cst.tile([P, (A + CB) * B], bf16)
            nc.sync.dma_start(out=iota_t[:], in_=iotas[:])
            iota_f = iota_t[:, 0 : A * B].rearrange("p (a g) -> p a g", a=A)
            iota_c = iota_t[:, A * B :].rearrange("p (c g) -> p c g", c=CB)

            ps = psp.tile([CB, A * F], f32)

            def body():
                gi = 0
                for t in range(n_tiles):
                    C = iop.tile([P, NPLANES * G], bf16, tag="C")
                    nc.sync.dma_start(out=C[:, 0 : 6 * G], in_=data[t])

                    def pl(i):
                        return C[:, i * G : (i + 1) * G]

                    x, y, vx, vy, vz, m = (pl(i) for i in range(6))

                    xx = fpp.tile([P, G], f32, tag="xx")
                    yy = fpp.tile([P, G], f32, tag="yy")
                    rsq = fpp.tile([P, G], f32, tag="rsq")
                    rcp = fpp.tile([P, G], f32, tag="rcp")
                    t5b = arp.tile([P, G], bf16, tag="t5b")
                    i5 = arp.tile([P, G], bf16, tag="i5")
                    ck = arp.tile([P, G], bf16, tag="ck")
                    t1 = arp.tile([P, G], bf16, tag="t1")
                    fk = arp.tile([P, G], bf16, tag="fk")
                    aa = arp.tile([P, G], bf16, tag="aa")
                    bb = arp.tile([P, G], bf16, tag="bb")
                    cc = arp.tile([P, G], bf16, tag="cc")
                    dd = arp.tile([P, G], bf16, tag="dd")
                    s = arp.tile([P, G], bf16, tag="s")
                    tt = arp.tile([P, G], bf16, tag="tt")
                    s2a = arp.tile([P, G], bf16, tag="s2a")
                    t2a = arp.tile([P, G], bf16, tag="t2a")
                    vz2a = arp.tile([P, G], bf16, tag="vz2a")
                    mi = arp.tile([P, G], bf16, tag="mi")
                    i2m = arp.tile([P, G], bf16, tag="i2m")

                    # --- binning keys -------------------------------------
                    nc.scalar.activation(xx[:], x, AF.Square)
                    nc.scalar.activation(yy[:], y, AF.Square)
                    nc.vector.tensor_tensor(out=rsq[:], in0=xx[:], in1=yy[:], op=AO.add)
                    nc.scalar.activation(t5b[:], rsq[:], AF.Sqrt, scale=25.0)
                    nc.vector.reciprocal_approx_fast(out=rcp[:], in_=rsq[:])
                    nc.scalar.activation(i5[:], rcp[:], AF.Sqrt, scale=0.04)
                    # ck = 192 + c where c = floor(t5/3);
                    # fk = t5 - 3c + 191.5 -> RNE -> 192 + f
                    nc.vector.tensor_scalar(
                        ck[:], t5b[:], 1.0 / 3.0, KEY0 - 0.5, AO.mult, AO.add
                    )
                    nc.vector.tensor_scalar(
                        t1[:], ck[:], -3.0, 3.0 * KEY0 + KEY0 - 0.5, AO.mult, AO.add
                    )
                    nc.vector.tensor_tensor(out=fk[:], in0=t5b[:], in1=t1[:], op=AO.add)

                    # --- velocity moments (bf16, scales folded on host) ---
                    nc.vector.tensor_tensor(out=aa[:], in0=x, in1=vx, op=AO.mult)
                    nc.vector.tensor_tensor(out=bb[:], in0=y, in1=vy, op=AO.mult)
                    nc.vector.tensor_tensor(out=cc[:], in0=y, in1=vx, op=AO.mult)
                    nc.vector.tensor_tensor(out=dd[:], in0=x, in1=vy, op=AO.mult)
                    nc.vector.tensor_tensor(out=s[:], in0=aa[:], in1=bb[:], op=AO.add)
                    nc.vector.tensor_tensor(out=tt[:], in0=cc[:], in1=dd[:], op=AO.subtract)
                    nc.scalar.activation(s2a[:], s[:], AF.Square)
                    nc.scalar.activation(t2a[:], tt[:], AF.Square)
                    nc.scalar.activation(vz2a[:], vz, AF.Square)
                    nc.vector.tensor_tensor(out=mi[:], in0=m, in1=i5[:], op=AO.mult)
                    nc.vector.tensor_tensor(out=i2m[:], in0=mi[:], in1=i5[:], op=AO.mult)
                    nc.vector.tensor_tensor(out=pl(6), in0=s[:], in1=mi[:], op=AO.mult)
                    nc.vector.tensor_tensor(out=pl(7), in0=s2a[:], in1=i2m[:], op=AO.mult)
                    nc.vector.tensor_tensor(out=pl(8), in0=tt[:], in1=mi[:], op=AO.mult)
                    nc.vector.tensor_tensor(out=pl(9), in0=t2a[:], in1=i2m[:], op=AO.mult)
                    nc.vector.tensor_tensor(out=pl(10), in0=m, in1=vz, op=AO.mult)
                    nc.vector.tensor_tensor(out=pl(11), in0=m, in1=vz2a[:], op=AO.mult)

                    wf = C[:, 5 * G : 12 * G].rearrange("p (f g) -> p f g", f=F)

                    # --- scatter ------------------------------------------
                    for j in range(NB):
                        gs = slice(j * B, (j + 1) * B)
                        ohf = ohp.tile([P, A * B], bf16, tag="ohf")
                        ohc = ohp.tile([P, CB * B], bf16, tag="ohc")
                        E = ohp.tile([P, A * F * B], bf16, tag="E")

                        fkb = fk[:, gs].unsqueeze(1).broadcast_to([P, A, B])
                        nc.vector.tensor_tensor(
                            out=ohf[:].rearrange("p (a g) -> p a g", a=A),
                            in0=fkb, in1=iota_f, op=AO.is_equal,
                        )
                        ckb = ck[:, gs].unsqueeze(1).broadcast_to([P, CB, B])
                        nc.vector.tensor_tensor(
                            out=ohc[:].rearrange("p (c g) -> p c g", c=CB),
                            in0=ckb, in1=iota_c, op=AO.is_equal,
                        )
                        wfb = wf[:, :, gs].unsqueeze(1).broadcast_to([P, A, F, B])
                        ohfb = (
                            ohf[:].rearrange("p (a g) -> p a g", a=A)
                            .unsqueeze(2).broadcast_to([P, A, F, B])
                        )
                        nc.vector.tensor_tensor(
                            out=E[:].rearrange("p (a f g) -> p a f g", a=A, f=F),
                            in0=wfb, in1=ohfb, op=AO.mult,
                        )

                        ohc_v = ohc[:].rearrange("p (c g) -> p c g", c=CB)
                        E_v = E[:].rearrange("p (x g) -> p x g", x=A * F)
                        for g in range(B):
                            nc.tensor.matmul(
                                out=ps[:],
                                lhsT=ohc_v[:, :, g],
                                rhs=E_v[:, :, g],
                                start=(gi == 0),
                                stop=(gi == n_mm - 1),
                            )
                            gi += 1

            if reps == 1:
                body()
            else:
                with tc.For_i(0, reps):
                    body()

            out_sb = cst.tile([CB, A * F], f32)
            nc.vector.tensor_copy(out=out_sb[:], in_=ps[:])
            nc.sync.dma_start(out=hist[:], in_=out_sb[:])

    nc.compile()
    return nc


LAST_RESULTS = None


def _pack_core(positions, velocities, masses, n_tiles):
    """f32 [npc,3]x2 + [npc] -> bf16 [T, 128, 6G] planar (x y vx vy vz m)."""
    out = np.empty((n_tiles, P, 6 * G), dtype=ml_dtypes.bfloat16)
    pr = positions.reshape(n_tiles, P, G, 3)
    vr = velocities.reshape(n_tiles, P, G, 3)
    out[:, :, 0 * G : 1 * G] = pr[:, :, :, 0]
    out[:, :, 1 * G : 2 * G] = pr[:, :, :, 1]
    out[:, :, 2 * G : 3 * G] = vr[:, :, :, 0]
    out[:, :, 3 * G : 4 * G] = vr[:, :, :, 1]
    out[:, :, 4 * G : 5 * G] = vr[:, :, :, 2]
    out[:, :, 5 * G : 6 * G] = masses.reshape(n_tiles, P, G)
    return out


def _iotas():
    it = np.empty(((A + CB) * B,), dtype=np.float32)
    for a in range(A):
        it[a * B : (a + 1) * B] = KEY0 + a
    for c in range(CB):
        it[(A + c) * B : (A + c + 1) * B] = KEY0 + c
    return np.tile(it[None, :], (P, 1)).astype(ml_dtypes.bfloat16)


def _postprocess(hsum):
    """hsum [CB, A*F] f64 -> kin [6, 50] f32."""
    h = hsum.reshape(CB, A, F).reshape(CB * A, F)[: A * CB - 1]  # drop idx 50
    mass = h[:, 0]
    with np.errstate(divide="ignore", invalid="ignore"):
        mm = h[:, 1:] / mass[:, None]
        vr_m = 5.0 * mm[:, 0]
        vr2 = 25.0 * mm[:, 1]
        vph_m = 5.0 * mm[:, 2]
        vph2 = 25.0 * mm[:, 3]
        vz_m = mm[:, 4]
        vz2 = mm[:, 5]
        vr_sig = np.sqrt(np.maximum(vr2 - vr_m**2, 0.0))
        vph_sig = np.sqrt(np.maximum(vph2 - vph_m**2, 0.0))
        vz_sig = np.sqrt(np.maximum(vz2 - vz_m**2, 0.0))
    return np.stack((vph_m, vph_sig, vr_m, vr_sig, vz_m, vz_sig)).astype(np.float32)


def kernel(positions, velocities, masses, trace=False):
    global LAST_RESULTS
    positions = np.ascontiguousarray(np.asarray(positions, dtype=np.float32))
    velocities = np.ascontiguousarray(np.asarray(velocities, dtype=np.float32))
    masses = np.ascontiguousarray(np.asarray(masses, dtype=np.float32))
    n = positions.shape[0]
    assert n % (N_CORES * P * G) == 0, n
    npc = n // N_CORES
    n_tiles = npc // (P * G)

    key = n_tiles
    if key not in _CACHE:
        _CACHE[key] = _build(n_tiles)
    nc = _CACHE[key]

    iota = _iotas()
    in_maps = []
    for k in range(N_CORES):
        sl = slice(k * npc, (k + 1) * npc)
        in_maps.append(
            {
                "data": _pack_core(
                    positions[sl], velocities[sl], masses[sl], n_tiles
                ),
                "iotas": iota,
            }
        )

    res = run_bass_kernel_spmd(
        nc, in_maps, core_ids=list(range(N_CORES)), trace=trace
    )
    LAST_RESULTS = res

    hsum = np.zeros((CB, A * F), dtype=np.float64)
    for r in res.results:
        hsum += r["hist"].astype(np.float64)
    return _postprocess(hsum)
